# revision 1
# baseline (speedup 1.0000x reference)
"""Trainium2 Bass kernel for nn_CrossAttnHead (cross-attention head + FFN), v2.

Math (reference):
  Q = concat(A bcast over t, phi_tar bcast over (b,h)) @ Wq^T + bq
  K,V = H_emb_obs @ {Wk,Wv}^T + b
  scores = (Qh . Kh)/sqrt(dh) ; attn = softmax(scores, axis=o)
  ctx = attn @ Vh ; y = Linear2(relu(LN(Linear1(ctx))))

Device structure (v2, bf16 matmuls):
  Q[b,h,t] = QA[b,h] + Qphi[t]  (concat-linear splits into two small matmuls)
  => exp(scores/s) = w[b,h,n,o] * U[b,n,t,o],  w = exp(SA/s), U = exp(Sphi/s)
  ctx (numerator | denominator) from one matmul of U against w-scaled
  (V|1|0) with a 34-wide d-block so the w*V product runs in DVE 2x mode
  (w pair-duplicated so every operand has a packed innermost dim).
  FFN: PE-transpose x1 -> x2 = x1t @ W1 with b1 planted in PSUM via a K=1
  matmul; grouped bn_stats; per-head LN+relu on Act; y-dot = DVE 2x mul +
  reduce against W2.

Sharding: data-parallel over B: 16 batches -> 8 cores x 2 batches.
"""

import os
import numpy as np
import ml_dtypes

import concourse.bass as bass
import concourse.mybir as mybir
import concourse.tile as tile
from concourse.bass_utils import run_bass_kernel_spmd
from concourse.masks import make_identity
from concourse.vector_clock import ScopedClock
import bass_rust

# ---- problem constants (hardcoded per contract) ----
B, H, NTAR, NOBS = 16, 12, 256, 512
DSITE = 128          # d (site embedding)
DTOT = 128           # d_tot == h_temporal
M_BASIS = 128        # phi basis dim
NH, DH = 4, 32       # heads
D2 = DH + 2          # 32 num + den col + zero pad col
NCORES = 8
BPC = B // NCORES    # batches per core = 2
SCALE = 1.0 / np.sqrt(DH)
LN_EPS = 1e-5

f32 = mybir.dt.float32
bf16 = mybir.dt.bfloat16
AF = mybir.ActivationFunctionType
ALU = mybir.AluOpType

OC = 4               # NOBS / 128 chunks
import os as _os
PSA_BUFS = int(_os.environ.get("PSA_BUFS", "5"))
PSC_BUFS = int(_os.environ.get("PSC_BUFS", "2"))
PSX_BUFS = int(_os.environ.get("PSX_BUFS", "1"))
PSX2_BUFS = int(_os.environ.get("PSX2_BUFS", "3"))
SHARE_X2 = int(_os.environ.get("SHARE_X2", "1"))
LASTY = int(_os.environ.get("LASTY", "0"))
DMASPLIT = int(_os.environ.get("DMASPLIT", "0"))
PIPE = int(_os.environ.get("PIPE", "1"))
Y_MODE = _os.environ.get("Y_MODE", "pe")
RELU_DVE = int(_os.environ.get("RELU_DVE", "1"))
VP_POOL = int(_os.environ.get("VP_POOL", "0"))
STATS_MODE = _os.environ.get("STATS_MODE", "ttr")   # "bn" | "ttr"
STATS_ACT = int(_os.environ.get("STATS_ACT", "2"))  # per-bank stats ops on Act (ttr mode)

# bf16 blob column offsets: WqPT|phiT|WqAT|AT|WkT|WvT|W1T|W2rep|ones+b1rep4(row0)
O_WQP, O_PHI, O_WQA, O_AT = 0, 128, 384, 512
O_WK, O_WV, O_W1, O_W2C, O_W2R, O_OB = 536, 664, 792, 920, 928, 1056
NB_COLS = O_OB + 640
# f32 blob: bq|bk|bv_rep|b2  (+ g_rep|lb_rep when ln_general)
NF_COLS = 131
TC = 2               # NTAR / 128 chunks
HQ = 3               # h-quads per tc (12 h / 4)


# ------------------------------------------------------------------
# walrus in this container rejects >1 sem wait per instruction (CTRL Drain,
# matmul LDWEIGHTS struct, ...). Two patches:
#  1) _add_instruction: hoist all-but-one waits of any instruction onto
#     same-engine NoOps inserted just before it (sequencer semantics are
#     identical: all waits must pass before the inst dispatches).
#  2) _drain_and_barrier: the tail drain gets its waits added after
#     insertion, so split it into one drain per wait.
def _install_drain_patch():
    if getattr(tile.TileContext, "_drain_patch_installed", False):
        return
    tile.TileContext._drain_patch_installed = True
    _orig_add = tile.TileContext._add_instruction

    def _add_split(self, inst):
        si = getattr(inst, "sync_info", None)
        if si is not None and si.on_wait and len(si.on_wait) > 1:
            waits = list(si.on_wait)
            si.on_wait = waits[-1:]
            for w in waits[:-1]:
                nop = mybir.InstNoOp(
                    name=self.nc.get_next_instruction_name(),
                    sync_info=mybir.SyncInfo(on_wait=[w], on_update=[]),
                    bass_nofuse=True,
                    engine=inst.engine,
                )
                _orig_add(self, nop)
        _orig_add(self, inst)

    tile.TileContext._add_instruction = _add_split

    def _patched(self, tick_clock, wait_clock):
        d0 = self.nc.sync.drain()
        wait_clock.add_sem_waits(
            d0.ins, ScopedClock({None: tick_clock.global_clock})
        )
        si = d0.ins.sync_info
        if si is not None and si.on_wait and len(si.on_wait) > 1:
            waits = list(si.on_wait)
            si.on_wait = waits[:1]
            for w in waits[1:]:
                d = self.nc.sync.drain()
                dsi = d.ins.sync_info
                if dsi is None:
                    d.ins.sync_info = bass_rust.SyncInfo(
                        on_wait=[w], on_update=[]
                    )
                else:
                    dsi.on_wait = [w]
        self.nc.all_engine_barrier()
        popped = self.nc._tile_sem_poison_stack.pop()
        assert popped is self._sem_poison
        # chunk the sem range-clears: wide EVENT_SEMAPHORE_RANGE_CLEAR
        # trips "ISA wrong length" in this walrus build
        sems = list(self.sems.allocated().values())
        for i in range(0, len(sems), 4):
            self.nc.clear_and_free_semaphores(sems[i : i + 4])
        self.nc.all_engine_barrier()

    tile.TileContext._drain_and_barrier = _patched


_install_drain_patch()


def _bc(ap, ap_dims, offset=None):
    """AP with the same tensor but explicit [step, count] dims."""
    return bass.AP(
        tensor=ap.tensor,
        offset=ap.offset if offset is None else offset,
        ap=ap_dims,
    )


def build_nc(ln_general: bool):
    nc = bass.Bass()

    # ---- DRAM I/O ----
    BLOB_B = nc.dram_tensor("BLOB_B", [128, NB_COLS], bf16, kind="ExternalInput")
    Hemb = nc.dram_tensor("Hemb", [BPC, NOBS, DSITE], bf16, kind="ExternalInput")
    nf = NF_COLS + (256 if ln_general else 0)
    BLOB_F = nc.dram_tensor("BLOB_F", [128, nf], f32, kind="ExternalInput")
    Y = nc.dram_tensor("Y", [BPC * TC * H, 128], f32, kind="ExternalOutput")

    with tile.TileContext(nc) as tc:
        _emit(nc, tc, locals(), ln_general)
    return nc


def _emit(nc, tc, T, ln_general):
    from contextlib import ExitStack

    ctxmgr = ExitStack()
    with ctxmgr:
        singles = ctxmgr.enter_context(tc.tile_pool(name="singles", bufs=1))
        sb_htb = ctxmgr.enter_context(tc.tile_pool(name="htb", bufs=2))
        sb_ktb = ctxmgr.enter_context(tc.tile_pool(name="ktb", bufs=2))
        sb_v1a = ctxmgr.enter_context(tc.tile_pool(name="v1a", bufs=5))
        sb_w = ctxmgr.enter_context(tc.tile_pool(name="wp", bufs=5))
        sb_vp = ctxmgr.enter_context(tc.tile_pool(name="vp", bufs=6))
        sb_ut = ctxmgr.enter_context(tc.tile_pool(name="ut", bufs=6))
        sb_x1 = ctxmgr.enter_context(tc.tile_pool(name="x1", bufs=4))
        sb_x1t = ctxmgr.enter_context(tc.tile_pool(name="x1t", bufs=5))
        sb_x3 = ctxmgr.enter_context(tc.tile_pool(name="x3", bufs=3))
        sb_x3t = ctxmgr.enter_context(tc.tile_pool(name="x3t", bufs=5))
        sb_st = ctxmgr.enter_context(tc.tile_pool(name="st", bufs=12))
        ps_a = ctxmgr.enter_context(tc.tile_pool(name="psa", bufs=PSA_BUFS, space="PSUM"))
        ps_c = ctxmgr.enter_context(tc.tile_pool(name="psc", bufs=PSC_BUFS, space="PSUM"))
        ps_x = ctxmgr.enter_context(tc.tile_pool(name="psx", bufs=PSX_BUFS, space="PSUM"))

        # ---- load constants: one bf16 blob + one f32 blob + Hemb per b ----
        blob_b = singles.tile([128, NB_COLS], bf16, name="blob_b", tag="blob_b")
        hemb_sb = []
        def _hemb_dma(b):
            ht = singles.tile([128, OC, 128], bf16, name=f"hemb{b}", tag=f"hemb{b}")
            hap = T["Hemb"][:]
            src_ap = _bc(
                hap,
                [[DSITE, 128], [128 * DSITE, OC], [1, DSITE]],
                offset=b * NOBS * DSITE,
            )
            nc.sync.dma_start(out=ht[:], in_=src_ap)
            hemb_sb.append(ht)
        bap = T["BLOB_B"][:]
        if DMASPLIT:
            _hemb_dma(0)
            nc.sync.dma_start(
                out=blob_b[:, 0:O_WK],
                in_=_bc(bap, [[NB_COLS, 128], [1, O_WK]]),
            )
            nc.sync.dma_start(
                out=blob_b[:, O_WK:],
                in_=_bc(bap, [[NB_COLS, 128], [1, NB_COLS - O_WK]], offset=O_WK),
            )
            _hemb_dma(1)
        else:
            nc.sync.dma_start(out=blob_b[:], in_=T["BLOB_B"][:])
            _hemb_dma(0)
            _hemb_dma(1)
        nf = NF_COLS + (256 if ln_general else 0)
        blob_f = singles.tile([128, nf], f32, name="blob_f", tag="blob_f")
        nc.sync.dma_start(out=blob_f[:], in_=T["BLOB_F"][:])

        wqpt_s = blob_b[:, O_WQP : O_WQP + 128]
        phiT_s = blob_b[:, O_PHI : O_PHI + 256]
        wqat_s = blob_b[:, O_WQA : O_WQA + 128]
        at_s = blob_b[:, O_AT : O_AT + BPC * H]
        wkt_s = blob_b[:, O_WK : O_WK + 128]
        wvt_s = blob_b[:, O_WV : O_WV + 128]
        w1t_s = blob_b[:, O_W1 : O_W1 + 128]
        w2c_s = blob_b[:, O_W2C : O_W2C + 1]
        w2r_s = blob_b[:, O_W2R : O_W2R + 128]
        ones_s = blob_b[0:1, O_OB : O_OB + 128]
        b1r_s = blob_b[0:1, O_OB + 128 : O_OB + 640]
        bq_s = blob_f[:, 0:1]
        bk_s = blob_f[:, 1:2]
        bv_s = blob_f[:, 2:130]
        b2_s = blob_f[:, 130:131]
        if ln_general:
            g_s = blob_f[:, 131:259]
            lb_s = blob_f[:, 259:387]
        ident = singles.tile([128, 128], bf16, name="ident", tag="ident")
        make_identity(nc, ident[:])
        ident32 = singles.tile([128, 128], f32, name="ident32", tag="ident32")
        make_identity(nc, ident32[:])
        eps_s = singles.tile([128, 1], f32, name="eps_s", tag="eps")
        nc.vector.memset(eps_s[:], LN_EPS)

        # ---- Qphi^T = WqP @ phi^T + bq : [j=128, t=256] (bf16) ----
        qphi_ps = ps_a.tile([128, NTAR], f32, name="qphi_ps", tag="ps")
        nc.tensor.matmul(qphi_ps[:], wqpt_s, phiT_s, start=True, stop=True)
        qphiT = singles.tile([128, NTAR], bf16, name="qphiT", tag="qphiT")
        nc.scalar.add(out=qphiT[:], in_=qphi_ps[:], add=bq_s)

        # ---- QA^T : [j=128, (b,h)=24] ----
        qa_ps = ps_a.tile([128, BPC * H], f32, name="qa_ps", tag="ps")
        nc.tensor.matmul(qa_ps[:], wqat_s, at_s, start=True, stop=True)
        qaT = singles.tile([128, BPC * H], bf16, name="qaT", tag="qaT")
        nc.scalar.copy(out=qaT[:], in_=qa_ps[:])

        # ---- head-masked (block-diagonal) Qphi / QA so per-head K=32
        # contractions become full-K=128 matmuls ----
        qblk = singles.tile([128, NH, NTAR], bf16, name="qblk", tag="qblk")
        nc.vector.memset(qblk[:], 0.0)
        qablk = singles.tile([128, BPC, NH, H], bf16, name="qablk", tag="qablk")
        nc.vector.memset(qablk[:], 0.0)
        for n in range(NH):
            nc.gpsimd.tensor_copy(
                out=qblk[32 * n : 32 * (n + 1), n, :],
                in_=qphiT[32 * n : 32 * (n + 1), :],
            )
            nc.gpsimd.tensor_copy(
                out=qablk[32 * n : 32 * (n + 1), :, n, :],
                in_=qaT[32 * n : 32 * (n + 1), :].rearrange(
                    "p (b h) -> p b h", b=BPC
                ),
            )

        y_all = singles.tile([128, BPC, TC, H], f32, name="y_all", tag="y_all")

        all_ut = {}
        all_vp = {}
        for b in range(BPC):
            # ---- H^T via PE transpose: htb [c=128, o=512] bf16 ----
            ht_ps = ps_a.tile([128, OC, 128], bf16, name=f"ht_ps{b}", tag="ps")
            for oc in range(OC):
                nc.tensor.transpose(ht_ps[:, oc, :], hemb_sb[b][:, oc, :], ident[:])
            htb = sb_htb.tile([128, NOBS], bf16, name=f"htb{b}", tag="htb")
            nc.vector.tensor_copy(out=htb[:], in_=ht_ps[:])

            # ---- K^T = Wk @ H^T + bk : ktb [j=128, o=512] bf16 ----
            kt_ps = ps_a.tile([128, NOBS], f32, name=f"kt_ps{b}", tag="ps")
            nc.tensor.matmul(kt_ps[:], wkt_s, htb[:], start=True, stop=True)
            ktb = sb_ktb.tile([128, NOBS], bf16, name=f"ktb{b}", tag="ktb")
            nc.scalar.add(out=ktb[:], in_=kt_ps[:], add=bk_s)

            # ---- per o-chunk: V -> v1a, SA -> w_pair, Sphi -> U, V' ----
            vp_tiles = []
            ut_tiles = []
            for oc in range(OC):
                ksl = ktb[:, oc * 128 : (oc + 1) * 128]

                # V chunk [o=128, c=128]
                v_ps = ps_a.tile([128, 128], f32, name=f"v_ps{b}_{oc}", tag="ps")
                nc.tensor.matmul(
                    v_ps[:],
                    htb[:, oc * 128 : (oc + 1) * 128],
                    wvt_s,
                    start=True,
                    stop=True,
                )
                # v1a [o, n, 34]: (V + bv | 1 | 0)
                v1a = sb_v1a.tile([128, NH, D2], bf16, name=f"v1a{b}_{oc}", tag="v1a")
                nc.vector.tensor_add(
                    out=v1a[:, :, 0:DH],
                    in0=v_ps[:].rearrange("p (n d) -> p n d", n=NH),
                    in1=bv_s.rearrange("p (n d) -> p n d", n=NH),
                )
                nc.gpsimd.memset(v1a[:, :, DH : DH + 1], 1.0)
                nc.gpsimd.memset(v1a[:, :, DH + 1 : D2], 0.0)

                # SA^T -> w_pair [o, n, h, 2] = exp(SA/s) written twice
                sa_ps = ps_a.tile([128, NH * H], f32, name=f"sa_ps{b}_{oc}", tag="ps")
                nc.tensor.matmul(
                    sa_ps[:],
                    ksl,
                    qablk[:, b, :, :],
                    start=True,
                    stop=True,
                )
                wp = sb_w.tile([128, NH, H, 2], bf16, name=f"wp{b}_{oc}", tag="wp")
                sa_ap = sa_ps[:]
                nc.scalar.activation(
                    out=wp[:],
                    in_=_bc(sa_ap, [sa_ap.ap[0], [H, NH], [1, H], [0, 2]]),
                    func=AF.Exp,
                    scale=SCALE,
                )

                # Sphi^T (-> U) [o, n, t=256] : two K=128 N=512 matmuls
                u_ps = ps_a.tile([128, 2, NTAR], f32, name=f"u_psA{b}_{oc}", tag="ps")
                u_ps2 = ps_a.tile([128, 2, NTAR], f32, name=f"u_psB{b}_{oc}", tag="ps")
                nc.tensor.matmul(u_ps[:], ksl, qblk[:, 0:2, :], start=True, stop=True)
                nc.tensor.matmul(u_ps2[:], ksl, qblk[:, 2:4, :], start=True, stop=True)
                ut = sb_ut.tile([128, NH, NTAR], bf16, name=f"ut{b}_{oc}", tag="ut")
                nc.scalar.activation(
                    out=ut[:, 0:2, :], in_=u_ps[:], func=AF.Exp, scale=SCALE
                )
                nc.scalar.activation(
                    out=ut[:, 2:4, :], in_=u_ps2[:], func=AF.Exp, scale=SCALE
                )
                ut_tiles.append(ut)

                # V' [o, n, h, 34] = v1a * w  (DVE 2x: both innermost packed)
                vp = sb_vp.tile([128, NH, H, D2], bf16, name=f"vp{b}_{oc}", tag="vp")
                v1a_ap = v1a[:]
                in0 = _bc(
                    v1a_ap,
                    [v1a_ap.ap[0], [D2, NH], [0, H], [2, D2 // 2], [1, 2]],
                )
                wp_ap = wp[:]
                in1 = _bc(
                    wp_ap,
                    [wp_ap.ap[0], [H * 2, NH], [2, H], [0, D2 // 2], [1, 2]],
                )
                vp_ap = vp[:]
                outv = _bc(
                    vp_ap,
                    [vp_ap.ap[0], [H * D2, NH], [D2, H], [2, D2 // 2], [1, 2]],
                )
                vp_eng = nc.gpsimd if (b * OC + oc) % 8 < VP_POOL else nc.vector
                vp_eng.tensor_mul(out=outv, in0=in0, in1=in1)
                vp_tiles.append(vp)
            all_ut[b] = ut_tiles
            all_vp[b] = vp_tiles

        # ---- per (b, tc): ctx accumulation + normalize + FFN ----
        deferred_y = []
        pending_y = []
        tail_phase = [None]
        for b in range(BPC):
            ut_tiles = all_ut[b]
            vp_tiles = all_vp[b]
            for tcc in range(TC):
                for fn in pending_y:
                    fn()
                del pending_y[:]
                x1 = sb_x1.tile([128, H, NH, DH], bf16, name=f"x1_{b}_{tcc}", tag="x1")
                for n in range(NH):
                    ctx_ps = (ps_a if SHARE_X2 > 2 else ps_c).tile(
                        [128, H, D2], f32, name=f"ctx{b}_{tcc}_{n}",
                        tag=("ps" if SHARE_X2 > 2 else "ctx"),
                        bufs=(PSA_BUFS if SHARE_X2 > 2 else None),
                    )
                    for oc in range(OC):
                        nc.tensor.matmul(
                            ctx_ps[:],
                            ut_tiles[oc][:, n, tcc * 128 : (tcc + 1) * 128],
                            vp_tiles[oc][:, n, :, :],
                            start=(oc == 0),
                            stop=(oc == OC - 1),
                        )
                    # x1[:, n, h, d] = num * (1/den)
                    cap = ctx_ps[:]
                    rec = sb_st.tile([128, H], f32, name=f"rec{b}_{tcc}_{n}", tag="rec")
                    nc.vector.reciprocal(
                        out=rec[:],
                        in_=_bc(cap, [cap.ap[0], [D2, H]], offset=cap.offset + DH),
                    )
                    rap = rec[:]
                    nc.vector.tensor_mul(
                        out=x1[:, :, n, :],
                        in0=ctx_ps[:, :, 0:DH],
                        in1=_bc(rap, [rap.ap[0], [1, H], [0, DH]]),
                    )

                # ---- FFN: per h-quad ----
                x3 = sb_x3.tile([128, H, 128], bf16, name=f"x3_{b}_{tcc}", tag="x3")
                # -- S1: all quads: transpose, evac, x2 matmuls --
                x2_banks = []
                x1_ap = x1[:]
                for hq in range(HQ):
                    xp_ps = (ps_a if SHARE_X2 > 1 else ps_x).tile(
                        [128, 4, 128], bf16, name=f"xp{b}_{tcc}_{hq}",
                        tag=("ps" if SHARE_X2 > 1 else "psx"),
                        bufs=(PSA_BUFS if SHARE_X2 > 1 else None),
                    )
                    for i in range(4):
                        h = hq * 4 + i
                        nc.tensor.transpose(xp_ps[:, i, :], x1[:, h, :, :], ident[:])
                    x1t = sb_x1t.tile(
                        [128, 4, 128], bf16, name=f"x1t{b}_{tcc}_{hq}", tag="x1t"
                    )
                    nc.vector.tensor_copy(out=x1t[:], in_=xp_ps[:])
                    x2_ps = ps_a.tile(
                        [128, 4, 128], f32, name=f"x2{b}_{tcc}_{hq}", tag="ps", bufs=PSA_BUFS
                    ) if SHARE_X2 else ps_x.tile(
                        [128, 4, 128], f32, name=f"x2{b}_{tcc}_{hq}", tag="psx2", bufs=PSX2_BUFS
                    )
                    nc.tensor.matmul(x2_ps[:], ones_s, b1r_s, start=True, stop=False)
                    for i in range(4):
                        nc.tensor.matmul(
                            x2_ps[:, i, :],
                            x1t[:, i, :],
                            w1t_s,
                            start=False,
                            stop=(i == 3),
                        )
                    x2_banks.append(x2_ps)

                # -- S2: all quads: LN stats --
                if PIPE and tail_phase[0] is not None:
                    tail_phase[0]()
                    tail_phase[0] = None

                def _phase2(b=b, tcc=tcc, x2_banks=x2_banks, x3=x3, x1=x1):
                    rstds = []
                    nmrs = []
                    for hq in range(HQ):
                        x2_ps = x2_banks[hq]
                        if STATS_MODE == "bn":
                            stats = sb_st.tile(
                                [128, 4, 6], f32, name=f"bst{b}_{tcc}_{hq}", tag="bst"
                            )
                            for i in range(4):
                                nc.vector.bn_stats(out=stats[:, i, :], in_=x2_ps[:, i, :])
                            mv = sb_st.tile(
                                [128, 4, 2], f32, name=f"mv{b}_{tcc}_{hq}", tag="mv"
                            )
                            for i in range(4):
                                nc.vector.bn_aggr(out=mv[:, i, :], in_=stats[:, i, :])
                            std = sb_st.tile([128, 4], f32, name=f"std{b}_{tcc}_{hq}", tag="std")
                            nc.scalar.activation(
                                out=std[:], in_=mv[:, :, 1], func=AF.Sqrt, bias=eps_s[:]
                            )
                            rstd = sb_st.tile([128, 4], f32, name=f"rstd{b}_{tcc}_{hq}", tag="rstd")
                            nc.vector.reciprocal(out=rstd[:], in_=std[:])
                            nmr = sb_st.tile([128, 4], f32, name=f"nmr{b}_{tcc}_{hq}", tag="nmr")
                            nc.vector.tensor_mul(out=nmr[:], in0=mv[:, :, 0], in1=rstd[:])
                            nc.vector.tensor_scalar_mul(out=nmr[:], in0=nmr[:], scalar1=-1.0)
                        elif hq < STATS_ACT:
                            ssq = sb_st.tile([128, 4], f32, name=f"ssq{b}_{tcc}_{hq}", tag="ssq")
                            for i in range(4):
                                junk = sb_st.tile(
                                    [128, 128], bf16, name=f"jk{b}_{tcc}_{hq}_{i}", tag="jk"
                                )
                                nc.scalar.activation(
                                    out=junk[:],
                                    in_=x2_ps[:, i, :],
                                    func=AF.Square,
                                    accum_out=ssq[:, i : i + 1],
                                )
                            std = sb_st.tile([128, 4], f32, name=f"std{b}_{tcc}_{hq}", tag="std")
                            nc.scalar.activation(
                                out=std[:], in_=ssq[:], func=AF.Sqrt,
                                bias=eps_s[:], scale=1.0 / 128.0,
                            )
                            rstd = sb_st.tile([128, 4], f32, name=f"rstd{b}_{tcc}_{hq}", tag="rstd")
                            nc.vector.reciprocal(out=rstd[:], in_=std[:])
                            nmr = None
                        else:
                            stats = sb_st.tile(
                                [128, 4, 6], f32, name=f"bst{b}_{tcc}_{hq}", tag="bst"
                            )
                            for i in range(4):
                                nc.vector.bn_stats(out=stats[:, i, :], in_=x2_ps[:, i, :])
                            mv = sb_st.tile(
                                [128, 4, 2], f32, name=f"mv{b}_{tcc}_{hq}", tag="mv"
                            )
                            for i in range(4):
                                nc.vector.bn_aggr(out=mv[:, i, :], in_=stats[:, i, :])
                            std = sb_st.tile([128, 4], f32, name=f"std{b}_{tcc}_{hq}", tag="std")
                            nc.scalar.activation(
                                out=std[:], in_=mv[:, :, 1], func=AF.Sqrt, bias=eps_s[:]
                            )
                            rstd = sb_st.tile([128, 4], f32, name=f"rstd{b}_{tcc}_{hq}", tag="rstd")
                            nc.vector.reciprocal(out=rstd[:], in_=std[:])
                            nmr = None
                        rstds.append(rstd)
                        nmrs.append(nmr)

                    # -- S3: all quads: LN + relu --
                    for hq in range(HQ):
                        x2_ps = x2_banks[hq]
                        rstd = rstds[hq]
                        nmr = nmrs[hq]
                        for i in range(4):
                            h = hq * 4 + i
                            if not ln_general:
                                if i < 4 - RELU_DVE:
                                    nc.scalar.activation(
                                        out=x3[:, h, :],
                                        in_=x2_ps[:, i, :],
                                        func=AF.Relu,
                                        scale=rstd[:, i : i + 1],
                                        bias=(0.0 if nmr is None else nmr[:, i : i + 1]),
                                    )
                                elif nmr is None:
                                    nc.vector.tensor_scalar(
                                        out=x3[:, h, :],
                                        in0=x2_ps[:, i, :],
                                        scalar1=rstd[:, i : i + 1],
                                        scalar2=0.0,
                                        op0=ALU.mult,
                                        op1=ALU.max,
                                    )
                                else:
                                    nc.vector.tensor_scalar(
                                        out=x3[:, h, :],
                                        in0=x2_ps[:, i, :],
                                        scalar1=rstd[:, i : i + 1],
                                        scalar2=nmr[:, i : i + 1],
                                        op0=ALU.mult,
                                        op1=ALU.add,
                                    )
                                    nc.vector.tensor_scalar_max(
                                        out=x3[:, h, :], in0=x3[:, h, :], scalar1=0.0
                                    )
                            else:
                                xn = sb_st.tile(
                                    [128, 128], f32, name=f"xn{b}_{tcc}_{hq}_{i}", tag="xn"
                                )
                                nc.scalar.activation(
                                    out=xn[:],
                                    in_=x2_ps[:, i, :],
                                    func=AF.Identity,
                                    scale=rstd[:, i : i + 1],
                                    bias=(0.0 if nmr is None else nmr[:, i : i + 1]),
                                )
                                nc.vector.tensor_mul(out=xn[:], in0=xn[:], in1=g_s)
                                nc.vector.tensor_add(out=xn[:], in0=xn[:], in1=lb_s)
                                nc.scalar.activation(
                                    out=x3[:, h, :], in_=xn[:], func=AF.Relu
                                )

                    # -- S4: y = x3 . W2 --
                    last_group = LASTY and (b == BPC - 1 and tcc == TC - 1)
                    if Y_MODE == "dve" or (Y_MODE == "pe" and last_group):
                        for hq in range(HQ):
                            scr = sb_x3t.tile(
                                [128, 4, 128], bf16, name=f"scr{b}_{tcc}_{hq}", tag="x3t"
                            )
                            nc.vector.tensor_mul(
                                out=scr[:],
                                in0=x3[:, hq * 4 : hq * 4 + 4, :],
                                in1=_bc(w2r_s, [w2r_s.ap[0], [0, 4], [1, 128]]),
                            )
                            nc.vector.tensor_reduce(
                                out=y_all[:, b, tcc, hq * 4 : hq * 4 + 4],
                                in_=scr[:],
                                axis=mybir.AxisListType.X,
                                op=ALU.add,
                            )
                    else:
                        def _emit_y(b=b, tcc=tcc, x3=x3):
                            for hq in range(HQ):
                                xq_ps = ps_x.tile(
                                    [128, 4, 128], bf16, name=f"xq{b}_{tcc}_{hq}", tag="psx"
                                )
                                for i in range(4):
                                    h = hq * 4 + i
                                    nc.tensor.transpose(
                                        xq_ps[:, i, :], x3[:, h, :], ident[:]
                                    )
                                x3t = sb_x3t.tile(
                                    [128, 4, 128], bf16, name=f"x3t{b}_{tcc}_{hq}", tag="x3t"
                                )
                                nc.vector.tensor_copy(out=x3t[:], in_=xq_ps[:])
                                yq_ps = ps_x.tile(
                                    [128, 4], f32, name=f"yq{b}_{tcc}_{hq}", tag="psx"
                                )
                                for i in range(4):
                                    nc.tensor.matmul(
                                        yq_ps[:, i : i + 1],
                                        x3t[:, i, :],
                                        w2c_s,
                                        start=True,
                                        stop=True,
                                    )
                                nc.vector.tensor_copy(
                                    out=y_all[:, b, tcc, hq * 4 : hq * 4 + 4], in_=yq_ps[:]
                                )
                        pending_y.extend(deferred_y)
                        deferred_y.clear()
                        deferred_y.append(_emit_y)

                if PIPE:
                    tail_phase[0] = _phase2
                else:
                    _phase2()

        if tail_phase[0] is not None:
            tail_phase[0]()
            tail_phase[0] = None
        for fn in pending_y + deferred_y:
            fn()

        # ---- finalize y: +b2, transpose, DMA out ----
        nc.gpsimd.tensor_scalar_add(out=y_all[:], in0=y_all[:], scalar1=b2_s)
        y_ps = ps_x.tile([BPC * TC * H, 128], f32, name="y_ps", tag="psx")
        nc.tensor.transpose(y_ps[:], y_all[:], ident32[:])
        yT = singles.tile([BPC * TC * H, 128], f32, name="yT", tag="yT")
        nc.scalar.copy(out=yT[:], in_=y_ps[:])
        nc.sync.dma_start(out=T["Y"][:], in_=yT[:])


# ------------------------------------------------------------------
def prepare_in_maps(inputs):
    bf = ml_dtypes.bfloat16
    A = np.ascontiguousarray(np.asarray(inputs["A"], np.float32))
    phi = np.ascontiguousarray(np.asarray(inputs["phi_tar"], np.float32))
    Hm = np.ascontiguousarray(np.asarray(inputs["H_emb_obs"], np.float32))
    Wq = np.asarray(inputs["Wq"], np.float32)
    Wk = np.asarray(inputs["Wk"], np.float32)
    Wv = np.asarray(inputs["Wv"], np.float32)
    W1 = np.asarray(inputs["W1"], np.float32)
    W2 = np.asarray(inputs["W2"], np.float32)
    b1 = np.asarray(inputs["b1"], np.float32)

    ln_g = np.asarray(inputs["ln_g"], np.float32)
    ln_b = np.asarray(inputs["ln_b"], np.float32)
    ln_general = not (
        np.allclose(ln_g, 1.0, atol=1e-7) and np.allclose(ln_b, 0.0, atol=1e-7)
    )

    if STATS_MODE == "ttr":
        W1u = W1 - W1.mean(axis=0, keepdims=True)
        b1u = b1 - b1.mean()
    else:
        W1u, b1u = W1, b1
    blob_b = np.zeros((128, NB_COLS), bf)
    blob_b[:, O_WQP : O_WQP + 128] = Wq[:, DTOT:].T.astype(bf)
    blob_b[:, O_PHI : O_PHI + 256] = phi.T.astype(bf)
    blob_b[:, O_WQA : O_WQA + 128] = Wq[:, :DTOT].T.astype(bf)
    blob_b[:, O_WK : O_WK + 128] = Wk.T.astype(bf)
    blob_b[:, O_WV : O_WV + 128] = Wv.T.astype(bf)
    blob_b[:, O_W1 : O_W1 + 128] = W1u.T.astype(bf)
    blob_b[:, O_W2C] = W2[0].astype(bf)
    blob_b[:, O_W2R : O_W2R + 128] = np.broadcast_to(W2[0], (128, 128)).astype(bf)
    blob_b[0, O_OB : O_OB + 128] = np.ones(128, bf)
    blob_b[0, O_OB + 128 : O_OB + 640] = np.tile(b1u, 4).astype(bf)

    nf = NF_COLS + (256 if ln_general else 0)
    blob_f = np.zeros((128, nf), np.float32)
    blob_f[:, 0] = np.asarray(inputs["bq"], np.float32)
    blob_f[:, 1] = np.asarray(inputs["bk"], np.float32)
    blob_f[:, 2:130] = np.broadcast_to(np.asarray(inputs["bv"], np.float32), (128, 128))
    blob_f[:, 130] = np.asarray(inputs["b2"], np.float32).reshape(())
    if ln_general:
        blob_f[:, 131:259] = np.broadcast_to(ln_g, (128, 128))
        blob_f[:, 259:387] = np.broadcast_to(ln_b, (128, 128))

    in_maps = []
    for k in range(NCORES):
        sl = slice(k * BPC, (k + 1) * BPC)
        bb = blob_b.copy()
        bb[:, O_AT : O_AT + BPC * H] = (
            A[sl].transpose(2, 0, 1).reshape(128, BPC * H).astype(bf)
        )
        m = {
            "BLOB_B": bb,
            "BLOB_F": blob_f,
            "Hemb": np.ascontiguousarray(Hm[sl]).astype(bf),
        }
        in_maps.append(m)
    return in_maps, ln_general


_nc_cache = {}
last_exec_time_ns = None
last_trace_path = None


def get_nc(ln_general: bool):
    if ln_general not in _nc_cache:
        _nc_cache[ln_general] = build_nc(ln_general)
    return _nc_cache[ln_general]


def kernel(**inputs) -> np.ndarray:
    global last_exec_time_ns, last_trace_path
    in_maps, ln_general = prepare_in_maps(inputs)
    nc = get_nc(ln_general)
    trace = bool(int(os.environ.get("KERNEL_TRACE", "0")))
    res = run_bass_kernel_spmd(
        nc, in_maps, core_ids=list(range(NCORES)), trace=trace
    )
    last_exec_time_ns = res.exec_time_ns
    if res.instructions_and_trace is not None:
        last_trace_path = res.instructions_and_trace[1]
    out = np.empty((B, H, NTAR, 1), np.float32)
    for k in range(NCORES):
        yk = np.asarray(res.results[k]["Y"], np.float32).reshape(BPC, TC, H, 128)
        out[k * BPC : (k + 1) * BPC, :, :, 0] = (
            yk.transpose(0, 2, 1, 3).reshape(BPC, H, NTAR)
        )
    return out

